# revision 37
# baseline (speedup 1.0000x reference)
"""AttentionPooling TRN2 kernel (fp16 streaming variant).

Math: for each batch b:
    scores = x_b @ W.T + bias            (N, ATT)
    logits = scores @ A.T                (N, M)   [as (M, N) transposed]
    weights = softmax(logits over N)
    out_b = weights @ x_b                (M, C)

Exact algebraic simplifications:
  * logits = x @ (A @ W).T + (A @ bias); the (A @ bias)[m] term is constant
    over N, so softmax cancels it -> bias drops out entirely.
  * G = A @ W (M, C) is precomputed on the host (tiny: 67 MFLOP), so the
    device only sees the N-scale work: logits = x @ G.T, softmax, pooling.

Dtype plan (sim rel err 2.4e-3 vs fp32 reference, tolerance 2e-2):
  * x ships as fp16: halves the HBM stream (8.4 MB/core vs 16.8) and runs
    PE transposes at 1.0 cyc/row (vs 1.5 for f32r).
  * gT fp16; logits accumulate in fp32 PSUM (PE always accumulates fp32).
  * E = exp(logits) stored bf16: fp32-like range (logits span +-44 here, so
    exp overflows fp16's 65504 ceiling), 8-bit mantissa only perturbs the
    softmax weights by ~4e-3 relative.
  * Softmax runs without max-subtraction: exp() in fp32->bf16 handles e^44.
  * Pooling matmul mixes bf16 lhsT (E^T) with fp16 rhs (x) into fp32 PSUM.

Sharding: data-parallel over B across the 8 cores (one batch each), no
collectives. Per core:
  - all 9 x-chunk DMAs issue up-front on the sync HWDGE ring (8.4 MB
    streams at ~400 GB/s with 4 KB descriptors, done by ~40 us -- well
    under the ~48 us PE schedule). Identities lead, gT follows chunk 0.
  - PE per chunk: 32 fp16 transposes (x -> xT), 8 logits matmuls
    (K=C tiles), 4 eT transposes (bf16), 8 pooling matmuls (K=n).
    Both pooling accumulators live in ONE psum bank on disjoint partition
    ranges, freeing a 4th transpose buffer (longer uninterrupted PE runs).
  - ACT does exp with accum_out producing the per-chunk row sums for free.
  - a 20-matmul warm-up plus interleaved high-MAC "kick" matmuls hold the
    HAM clock governor at 2.4 GHz through the transpose-heavy early phase.
  - after all chunks: scale rows by 1/sum (DVE h0 / ACT h1 in parallel),
    both halves DMA out via the fast sync ring (f32).

Measured: 66.5-67.3 us HW exec (baseline f32r kernel: 88.9 us); rel err
3.2e-3 vs the fp32 reference (tolerance 2e-2).
"""

import ml_dtypes
import numpy as np

import concourse.bacc as bacc
import concourse.mybir as mybir
import concourse.tile as tile
from concourse.bass_utils import run_bass_kernel_spmd

B, N, C = 8, 4096, 1024
ATT, M = 512, 64
NCORES = 8
CT = C // 128  # 8 c-tiles

F32 = mybir.dt.float32
F16 = mybir.dt.float16
BF16 = mybir.dt.bfloat16

Exp = mybir.ActivationFunctionType.Exp
AX = mybir.AxisListType
ALU = mybir.AluOpType

# chunk row counts: short first chunk so the PE transpose stream starts as
# soon as 0.5 MB has landed; short last chunk to shorten the end-of-kernel
# dependency tail.
SIZES = [256] + [512] * 7 + [256]
ROW0 = [sum(SIZES[:k]) for k in range(len(SIZES))]
NCH = len(SIZES)


def build_nc():
    nc = bacc.Bacc("TRN2", target_bir_lowering=False, debug=False)

    # x ships with TWO consecutive n-rows packed per 4 KB DMA row (fp16 rows
    # alone are 2 KB, which caps the SDMA engines at ~260 GB/s; 4 KB
    # descriptors run at line rate). Softmax+pooling are invariant to the
    # induced n-permutation: every consumer below indexes the same SBUF
    # tiles, so the permutation cancels.
    x_d = nc.dram_tensor("x", [N // 2, 2 * C], F16, kind="ExternalInput")
    g_d = nc.dram_tensor("gt", [C, M], F16, kind="ExternalInput")
    idf_d = nc.dram_tensor("idf", [128, 128], F16, kind="ExternalInput")
    idb_d = nc.dram_tensor("idb", [128, 128], BF16, kind="ExternalInput")
    o_d = nc.dram_tensor("o", [M, C], F32, kind="ExternalOutput")

    with tile.TileContext(nc) as tc:
        with (
            tc.tile_pool(name="const", bufs=1) as constp,
            tc.tile_pool(name="xpool", bufs=NCH) as xpool,
            tc.tile_pool(name="xtp", bufs=2) as xtp,
            tc.tile_pool(name="small", bufs=2) as smallp,
            tc.tile_pool(name="outp", bufs=1) as outp,
            tc.tile_pool(name="psT", bufs=4, space="PSUM") as psT,
            tc.tile_pool(name="psL", bufs=2, space="PSUM") as psL,
            tc.tile_pool(name="psE", bufs=1, space="PSUM") as psE,
            tc.tile_pool(name="psO", bufs=1, space="PSUM") as psO,
        ):
            # x chunks all issue immediately on the sync ring; chunk 0 is in
            # front so PE work can start ~2 us in. One batched DMA per chunk
            # (0.5-1 MB, 2 KB contiguous rows) keeps the SDMA engines at
            # line rate; [p, s, c] lands 128-row tiles side by side in SBUF.
            x_re = x_d.ap().rearrange("(s p) d -> p s d", p=128)

            # identities first on the sync ring (64 KB, ~0.2 us): the chunk-0
            # transposes stream idf as their moving operand, and the scalar
            # ring is far too slow (~29 GB/s) to deliver it in time
            idf_sb = constp.tile([128, 128], F16)
            nc.sync.dma_start(idf_sb[:], idf_d.ap())
            idb_sb = constp.tile([128, 128], BF16)
            nc.sync.dma_start(idb_sb[:], idb_d.ap())

            x_chunks = []
            for k in range(NCH):
                sub2 = SIZES[k] // 256  # packed slots (256 n-rows each)
                b0 = ROW0[k] // 256
                xt_ = xpool.tile([128, sub2, 2 * C], F16, tag="x", name=f"x_{k}")
                nc.sync.dma_start(xt_[:], x_re[:, b0 : b0 + sub2, :])
                # each packed slot holds two interleaved [128, C] n-tiles
                x_chunks.append(
                    [
                        xt_[:, s2, C * h : C * (h + 1)]
                        for s2 in range(sub2)
                        for h in range(2)
                    ]
                )
                if k == 0:
                    # gT rides the sync ring right behind chunk 0 (0.25 MB,
                    # lands ~1 us after it -- well before the first logits
                    # matmul needs it)
                    gT_sb = constp.tile([128, CT, M], F16)
                    nc.sync.dma_start(
                        gT_sb[:], g_d.ap().rearrange("(t p) m -> p t m", p=128)
                    )

            # HAM warm-up: throwaway matmuls so the real pipeline runs at
            # 2.4 GHz. A memset-created tile needs no DMA, so warming starts
            # the moment the preamble barrier clears, covering chunk 0's
            # DMA latency.
            warm_in = constp.tile([128, 256], F16, name="warm_in")
            nc.vector.memset(warm_in[:], 1.0)
            # [128, 256] fp32 keeps the psT pool slot at 1 KB/partition so
            # the pool stays within its PSUM banks
            # 20 matmuls: the HAM clock governor needs ~3.5 us of sustained
            # full-array MAC activity before it leaves half-speed, and the
            # transposes that open each chunk barely light up the array. A
            # long warm-up means chunk 0 starts at 2.4 GHz.
            warm_ps = psT.tile([128, 256], F32, tag="pst", name="warm_ps")
            for r in range(20):
                nc.tensor.matmul(
                    warm_ps[:], warm_in[:, :128], warm_in[:],
                    start=(r % 10 == 0), stop=(r % 10 == 9),
                )
            warm_out = constp.tile([128, 256], F32, name="warm_out")
            nc.vector.tensor_copy(warm_out[:], warm_ps[:])

            # high-MAC kick matmuls interleaved into the transpose-heavy
            # early chunks so the clock governor doesn't fall back to
            # half-speed before the steady matmul mix takes over. Each kick
            # takes a fresh psT rotation slot (1 KB, same as a pst tile).
            _kick_n = [0]

            def ham_kick():
                kp = psT.tile(
                    [128, 256], F32, tag="pst", name=f"kick_{_kick_n[0]}"
                )
                _kick_n[0] += 1
                nc.tensor.matmul(kp[:], warm_in[:, :128], warm_in[:])

            # per-chunk softmax row sums; last chunk splits into 2 slices
            sums_sb = outp.tile([M, NCH + 1], F32)
            # both pooling accumulators share ONE psum bank on disjoint
            # partition ranges: h0 -> partitions 0-63, h1 -> 64-127 (the
            # matmul's tile_position handles the 64-row offset). This frees
            # a bank for a 4th transpose buffer.
            psOut_t = psO.tile([128, 512], F32, name="psOut")
            psOut = [psOut_t[0:M, :], psOut_t[M : 2 * M, :]]

            def chunk_tail(k, e_sb, x_tiles):
                # E^T via PE transpose (bf16), then pooling accumulate.
                # Last chunk goes h-major so psOut[0] finishes early and its
                # scale+store overlaps psOut[1]'s remaining matmuls.
                sub = len(x_tiles)
                pse = psE.tile([128, sub * M], BF16, tag="pse", name=f"pse_{k}")
                for i in range(sub):
                    nc.tensor.transpose(
                        pse[:, M * i : M * (i + 1)],
                        e_sb[:, 128 * i : 128 * (i + 1)],
                        idb_sb[:M, :M],
                    )
                eT_sb = smallp.tile([128, sub * M], BF16, tag="et", name=f"eT_{k}")
                nc.vector.tensor_copy(eT_sb[:], pse[:])
                last = k == NCH - 1
                order = (
                    [(i, h) for h in range(C // 512) for i in range(sub)]
                    if last
                    else [(i, h) for i in range(sub) for h in range(C // 512)]
                )
                for i, h in order:
                    nc.tensor.matmul(
                        psOut[h],
                        eT_sb[:, M * i : M * (i + 1)],
                        x_tiles[i][:, 512 * h : 512 * (h + 1)],
                        start=(k == 0 and i == 0),
                        stop=(last and i == sub - 1),
                    )

            prev = None
            for k in range(NCH):
                x_tiles = x_chunks[k]
                nrows = SIZES[k]
                sub = nrows // 128

                # previous chunk's eT+pooling first: its inputs are already
                # ready, so the PE runs it as one contiguous block instead of
                # interleaving with the transposes (each T<->matmul mode
                # transition costs ~100-200 ns of pipeline flush)
                if prev is not None:
                    chunk_tail(*prev)

                xT = xtp.tile([128, CT * nrows], F16, tag="xt", name=f"xT_{k}")
                for j in range(CT):
                    pst = psT.tile([128, nrows], F16, tag="pst", name=f"pst_{k}_{j}")
                    for i in range(sub):
                        nc.tensor.transpose(
                            pst[:, 128 * i : 128 * (i + 1)],
                            x_tiles[i][:, 128 * j : 128 * (j + 1)],
                            idf_sb[:],
                        )
                    # split the PSUM drains between DVE and the mostly-idle
                    # scalar engine (gpsimd has no PSUM port)
                    dst = xT[:, nrows * j : nrows * (j + 1)]
                    if j % 4 != 3:
                        nc.vector.tensor_copy(dst, pst[:])
                    else:
                        nc.scalar.copy(dst, pst[:])
                    if k in (1, 2) and j % 2 == 1:
                        ham_kick()


                psl = psL.tile([M, nrows], F32, tag="psl", name=f"psl_{k}")
                for j in range(CT):
                    nc.tensor.matmul(
                        psl[:],
                        gT_sb[:, j, :],
                        xT[:, nrows * j : nrows * (j + 1)],
                        start=(j == 0),
                        stop=(j == CT - 1),
                    )

                # exp on ACT; accum_out produces the per-chunk row sum free
                e_sb = smallp.tile([M, nrows], BF16, tag="e", name=f"e_{k}")
                if k == NCH - 1:
                    # last chunk: exp per 128-col slice so the eT transposes
                    # and pooling start on slice 0 while slice 1 is still in
                    # the activation pipe (shortens the end-of-kernel chain).
                    # Row sums go to DVE here: accum_out's READ_ACCUMULATOR
                    # serializes the scalar engine for ~280 ns per slice,
                    # and this chain is the critical path to the output.
                    for i in range(nrows // 128):
                        nc.scalar.activation(
                            e_sb[:, 128 * i : 128 * (i + 1)],
                            psl[:, 128 * i : 128 * (i + 1)],
                            Exp,
                        )
                        nc.vector.tensor_reduce(
                            sums_sb[:, k + i : k + i + 1],
                            e_sb[:, 128 * i : 128 * (i + 1)],
                            axis=AX.X,
                            op=ALU.add,
                        )
                else:
                    nc.scalar.activation(
                        e_sb[:], psl[:], Exp,
                        accum_out=sums_sb[:, k : k + 1],
                    )

                prev = (k, e_sb, x_tiles)

            # total/recip depend only on the per-chunk sums -- issue before
            # the last chunk's pooling so DVE computes them under the PE work
            total = outp.tile([M, 1], F32)
            nc.vector.tensor_reduce(total[:], sums_sb[:], axis=AX.X, op=ALU.add)
            recip = outp.tile([M, 1], F32)
            nc.vector.reciprocal(recip[:], total[:])
            # replicate recip onto partitions 64-127 (where psOut h1 lives):
            # engines can't shift partitions, but an SBUF->SBUF DMA can, and
            # this 256 B transfer is far off the critical path
            rec_hi = outp.tile([2 * M, 1], F32, name="rec_hi")
            nc.sync.dma_start(rec_hi[M : 2 * M, :], recip[:])

            chunk_tail(*prev)

            # per-half scale + store: half 0 drains while half 1's pooling
            # matmuls are still running on the PE
            out_sb = outp.tile([M, C], F32)
            nc.vector.tensor_scalar_mul(out_sb[:, 0:512], psOut[0], recip[:])
            nc.sync.dma_start(o_d.ap()[:, 0:512], out_sb[:, 0:512])
            # half 1 scales on the scalar engine in-place on partitions
            # 64-127 (parallel with half 0's DVE scale); the DMA then maps
            # partitions 64-127 to output columns 512-1023. BOTH halves
            # store via the fast sync HWDGE ring -- the scalar ring runs at
            # ~29 GB/s and would add ~4 us of tail.
            out_hi = outp.tile([2 * M, 512], F32, name="out_hi")
            nc.scalar.activation(
                out_hi[M : 2 * M, :], psOut[1],
                mybir.ActivationFunctionType.Copy, scale=rec_hi[M : 2 * M, :],
            )
            nc.sync.dma_start(o_d.ap()[:, 512:1024], out_hi[M : 2 * M, :])

    nc.compile()
    return nc


_CACHE = {}


def _get_nc():
    if "nc" not in _CACHE:
        _CACHE["nc"] = build_nc()
    return _CACHE["nc"]


def _in_maps(x, W, attention_vectors):
    G = (np.asarray(attention_vectors, np.float32) @ np.asarray(W, np.float32))
    gt = np.ascontiguousarray(G.T).astype(np.float16)
    idf = np.eye(128, dtype=np.float16)
    idb = np.eye(128, dtype=ml_dtypes.bfloat16)
    x16 = np.asarray(x, np.float32).astype(np.float16)
    return [
        {
            "x": np.ascontiguousarray(x16[i]).reshape(N // 2, 2 * C),
            "gt": gt,
            "idf": idf,
            "idb": idb,
        }
        for i in range(x.shape[0])
    ]


def _run(x, W, attention_vectors, **spmd_kwargs):
    nc = _get_nc()
    return run_bass_kernel_spmd(
        nc, _in_maps(x, W, attention_vectors), core_ids=list(range(NCORES)),
        **spmd_kwargs,
    )


def kernel(x, W, b, attention_vectors):
    del b  # softmax over N cancels the (A @ b)[m] logit offset exactly
    x = np.asarray(x, dtype=np.float32)
    br = _run(x, np.asarray(W), np.asarray(attention_vectors))
    return np.stack([r["o"] for r in br.results], axis=0)
